# revision 1
# baseline (speedup 1.0000x reference)
"""Trainium2 Bass kernel for nn_EnhanceSelfAttention (B=16, N=577, C=768, H=12).

Self-contained: takes full unsharded inputs, shards batch across 8 NeuronCores
(2 batches/core), runs a fused attention kernel per core, gathers the output.

Per-core pipeline (fp16 matmul operands, fp32 PSUM accumulation):
  A. xT = x.T via PE transposes                       6 chunks [128, 1154] f16
  C. v = x @ qkv_w[:, v] + b, stored per k-tile in [k, 12*65] f16 layout
     with a ones column per head (softmax-denominator trick)
  G. gather exp(bias): the relative-position bias is read from a
     host-expanded [kh, dqw, qh, head] view of pos_emb (the Toeplitz
     structure of rel_index turns the [H,N,N] gather into 24 strided DMAs
     with 1.1KB contiguous lines; the causal -65504 mask is pre-folded into
     the table), then exp() on ScalarE -> per-k-tile f16 exp(bias) tiles
     (masked entries become exactly 0), laid out head-major.
  B. qT,kT = qkv_w[:, q|k].T @ xT, interleaved per head-pair with
  D. attention: sT = kT.T@qT (two heads packed into PE row groups, d=64),
     p = exp(sT) * expbias (no max-subtraction; scores are O(6)),
     OT += v.T@p accumulated over causal k-tiles in PSUM. Row 64 of OT is
     the softmax denominator; divide via DVE reciprocal + GpSimd
     partition-broadcast.
  E. y = OT.T @ out_w + out_b, streamed to DRAM.
"""

import numpy as np
import ml_dtypes

import concourse.bass as bass
import concourse.tile as tile
from concourse import bacc, mybir
from concourse.bass_utils import run_bass_kernel_spmd
from concourse.masks import make_identity

F32 = mybir.dt.float32
BF16 = mybir.dt.bfloat16
F16 = mybir.dt.float16

B, NTOK, CDIM, NH, DH = 16, 577, 768, 12, 64
GRID = 24
NRD = (2 * GRID - 1) * (2 * GRID - 1) + 3  # 2212
NCORES = 8
BLOC = B // NCORES       # batches per core
NSEQ = BLOC * NTOK       # 1154
SCALE = DH ** -0.5       # 0.125
NEG = -65504.0

QBLOCKS = [(0, 121), (121, 456)]            # (qstart, qN)
# k-tiles: (k0, pw).  t=0: partitions 0..120 <-> k=0..120 (incl cls col k=0)
KTILES = [(0, 121), (121, 120), (241, 120), (361, 120), (481, 96)]
# per-tile stored q range: [QLO[t] .. 577)
QLO = [0, 121, 241, 361, 481]
WID = [NTOK - q for q in QLO]               # 577, 456, 336, 216, 96

# expanded-table strides (T3m[kh, d1, qh, h], d1 = qw-kw+23)
T3_KH = 47 * GRID * NH   # 13536
T3_D1 = GRID * NH        # 288

_CACHE = {}


def _check_rel_index(ri):
    """Assert the Toeplitz structure the gather DMAs rely on."""
    assert ri.shape == (NTOK, NTOK)
    assert ri[0, 0] == NRD - 1
    assert (ri[0, 1:] == NRD - 3).all()
    assert (ri[1:, 0] == NRD - 2).all()
    a = np.arange(NTOK - 1)
    qh, qw = a % GRID, a // GRID
    rel0 = qh[:, None] - qh[None, :] + GRID - 1
    rel1 = qw[:, None] - qw[None, :] + GRID - 1
    expect = rel0 + rel1 * (2 * GRID - 1)
    assert np.array_equal(ri[1:, 1:], expect), "rel_index lacks expected structure"


def _host_prep(pos_emb, rel_index):
    _check_rel_index(np.asarray(rel_index))
    pe_t = np.asarray(pos_emb, dtype=np.float32).T      # [NRD, NH]
    # expanded gather table with the causal mask folded in:
    # T3m[kh, d1, qh, h] = pos_emb[h, (qh-kh+23) + 47*d1] + (NEG if q<k)
    # where q-k = (qh-kh) + 24*(d1-23)
    kh = np.arange(GRID)[:, None, None]
    d1 = np.arange(2 * GRID - 1)[None, :, None]
    qh = np.arange(GRID)[None, None, :]
    ridx = (qh - kh + GRID - 1) + (2 * GRID - 1) * d1   # [24, 47, 24]
    t3m = pe_t[ridx]                                    # [24, 47, 24, NH]
    masked = (qh - kh) + GRID * (d1 - (GRID - 1)) < 0
    t3m = t3m + np.where(masked, NEG, 0.0)[..., None].astype(np.float32)
    t3m = np.ascontiguousarray(t3m.reshape(-1)).astype(ml_dtypes.bfloat16)
    pos_embT = np.ascontiguousarray(pe_t).reshape(-1)
    return t3m, pos_embT


def _build(ri):
    """Build + compile the per-core Bass program."""
    nc = bacc.Bacc("TRN2", target_bir_lowering=False, debug=False)

    x_d = nc.dram_tensor("x_in", [NSEQ, CDIM], F32, kind="ExternalInput").ap()
    qkvwh_d = nc.dram_tensor("qkv_w_h", [CDIM, 3 * CDIM], F16,
                             kind="ExternalInput").ap()
    qkvb_d = nc.dram_tensor("qkv_b", [3 * CDIM], F32, kind="ExternalInput").ap()
    qkvbh_d = nc.dram_tensor("qkv_b_h", [3 * CDIM], F16, kind="ExternalInput").ap()
    t3m_d = nc.dram_tensor("t3m", [GRID * 47 * GRID * NH], BF16,
                           kind="ExternalInput").ap()
    pe_d = nc.dram_tensor("pos_embT", [NRD * NH], F32, kind="ExternalInput").ap()
    outwh_d = nc.dram_tensor("out_w_h", [CDIM, CDIM], F16,
                             kind="ExternalInput").ap()
    outbh_d = nc.dram_tensor("out_b_h", [CDIM], F16, kind="ExternalInput").ap()
    y_d = nc.dram_tensor("y", [NSEQ, CDIM], F32, kind="ExternalOutput").ap()

    with tile.TileContext(nc) as tc:
        _emit(nc, tc, ri, x_d, qkvwh_d, qkvb_d, qkvbh_d, t3m_d, pe_d,
              outwh_d, outbh_d, y_d)
    nc.compile()
    return nc


def _emit(nc, tc, ri, x_d, qkvw_d, qkvb_d, qkvbh_d, t3m_d, pe_d,
          outw_d, outbh_d, y_d):
    from contextlib import ExitStack

    NBLK = [(0, 386), (386, 384), (770, 384)]   # n-blocks for projections

    with ExitStack() as top:
        persist = top.enter_context(tc.tile_pool(name="persist", bufs=1))
        consts = top.enter_context(tc.tile_pool(name="consts", bufs=1))

        # ---- constants ----
        identity = consts.tile([128, 128], F32, tag="identity", name="identity")
        make_identity(nc, identity[:])
        identity_h = consts.tile([128, 128], F16, tag="identity_h",
                                 name="identity_h")
        nc.vector.tensor_copy(identity_h[:], identity[:])
        ones_f32 = consts.tile([1, 128], F32, tag="ones_f32", name="ones_f32")
        nc.vector.memset(ones_f32[:], 1.0)
        ones128 = consts.tile([1, 128], F16, tag="ones128", name="ones128")
        nc.vector.tensor_copy(ones128[:], ones_f32[:])

        # per-chunk qkv bias columns for q/k (q pre-scaled by SCALE)
        qk_bias = []
        for r in range(12):
            ch0 = r * 128 if r < 6 else CDIM + (r - 6) * 128
            braw = consts.tile([128, 1], F32, tag=f"qkb_raw{r}", name=f"qkb_raw{r}")
            nc.sync.dma_start(braw[:],
                              bass.AP(qkvb_d.tensor, ch0, [[1, 128], [1, 1]]))
            if r < 6:
                bsc = consts.tile([128, 1], F32, tag=f"qkb_sc{r}", name=f"qkb_sc{r}")
                nc.scalar.mul(bsc[:], braw[:], SCALE)
                qk_bias.append(bsc)
            else:
                qk_bias.append(braw)

        # broadcast bias rows: v-part of qkv_b, and out_b -> [128, 768] tiles
        with tc.tile_pool(name="bb_psum", bufs=2, space="PSUM") as bbps, \
             tc.tile_pool(name="bb_row", bufs=2) as bbrow:
            vb_row = bbrow.tile([1, CDIM], F16, tag="vb_row", name="vb_row")
            nc.sync.dma_start(vb_row[:], qkvbh_d[2 * CDIM:3 * CDIM].unsqueeze(0))
            ob_row = bbrow.tile([1, CDIM], F16, tag="ob_row", name="ob_row")
            nc.sync.dma_start(ob_row[:], outbh_d[:].unsqueeze(0))
            vbias = consts.tile([128, CDIM], F32, tag="vbias", name="vbias")
            obias = consts.tile([128, CDIM], F32, tag="obias", name="obias")
            for row, dst in ((vb_row, vbias), (ob_row, obias)):
                for h0, hw in ((0, 384), (384, 384)):
                    ps = bbps.tile([128, 384], F32, tag="bb", name="bb")
                    nc.tensor.matmul(ps[:], ones128[:], row[0:1, h0:h0 + hw],
                                     start=True, stop=True)
                    nc.vector.tensor_copy(dst[:, h0:h0 + hw], ps[:])

        # ---- persistent activation storage (all f16) ----
        qT = [persist.tile([128, NSEQ], F16, tag=f"qT{j}", name=f"qT{j}")
              for j in range(6)]
        kT = [persist.tile([128, NSEQ], F16, tag=f"kT{j}", name=f"kT{j}")
              for j in range(6)]
        vt = [[persist.tile([128, NH * 65], F16, tag=f"v{b}_{t}", name=f"v{b}_{t}")
               for t in range(5)] for b in range(BLOC)]
        oT = [persist.tile([128, NSEQ], F16, tag=f"oT{j}", name=f"oT{j}")
              for j in range(6)]
        expb = [persist.tile([128, WID[t] * NH], F16, tag=f"expb{t}",
                             name=f"expb{t}") for t in range(5)]

        # ================= phase A: x loads + xT =================
        with tc.tile_pool(name="xT", bufs=1) as xTp, \
             tc.tile_pool(name="wqk_pool", bufs=1) as wqk_pool:
            xT = [xTp.tile([128, NSEQ], F16, tag=f"xT{c}", name=f"xT{c}")
                  for c in range(6)]
            with tc.tile_pool(name="xload", bufs=8) as xload, \
                 tc.tile_pool(name="wv_pool", bufs=1) as wv_pool, \
                 tc.tile_pool(name="ps_t", bufs=6, space="PSUM") as ps_t, \
                 tc.tile_pool(name="ps_v", bufs=2, space="PSUM") as ps_v, \
                 tc.tile_pool(name="stage", bufs=3) as stagep:
                for m0 in range(0, NSEQ, 128):
                    mw = min(128, NSEQ - m0)
                    xt_in = xload.tile([128, CDIM], F16, tag="x", name="x")
                    nc.gpsimd.dma_start(xt_in[0:mw, :], x_d[m0:m0 + mw, :])
                    for c in range(6):
                        pt = ps_t.tile([128, 128], F16, tag="pt", name="pt")
                        nc.tensor.transpose(pt[0:128, 0:mw],
                                            xt_in[0:mw, c * 128:(c + 1) * 128],
                                            identity_h[0:mw, 0:mw])
                        if c % 2 == 0:
                            nc.vector.tensor_copy(xT[c][:, m0:m0 + mw],
                                                  pt[0:128, 0:mw])
                        else:
                            nc.scalar.copy(xT[c][:, m0:m0 + mw], pt[0:128, 0:mw])

                # ---- phase C: v ----
                wv = [wv_pool.tile([128, CDIM], F16, tag=f"wv{c}", name=f"wv{c}")
                      for c in range(6)]
                for c in range(6):
                    nc.sync.dma_start(wv[c][:],
                                      qkvw_d[c * 128:(c + 1) * 128,
                                             2 * CDIM:3 * CDIM])
                for b in range(BLOC):
                    for t, (k0, pw) in enumerate(KTILES):
                        vtile = vt[b][t]
                        for half in range(2):
                            ps = ps_v.tile([128, 384], F32, tag="psv", name="psv")
                            for c in range(6):
                                nc.tensor.matmul(
                                    ps[0:pw, :],
                                    xT[c][:, b * NTOK + k0:b * NTOK + k0 + pw],
                                    wv[c][:, half * 384:(half + 1) * 384],
                                    start=(c == 0), stop=(c == 5))
                            dst = vtile[0:pw, :].rearrange(
                                "p (h d) -> p h d", h=NH)[:, half * 6:(half + 1) * 6,
                                                          0:64]
                            src = ps[0:pw, :].rearrange("p (h d) -> p h d", d=64)
                            bsl = vbias[0:pw, half * 384:(half + 1) * 384].rearrange(
                                "p (h d) -> p h d", d=64)
                            nc.vector.tensor_tensor(out=dst, in0=src, in1=bsl,
                                                    op=mybir.AluOpType.add)
                        nc.vector.memset(
                            vtile[0:pw, :].rearrange("p (h d) -> p h d",
                                                     h=NH)[:, :, 64:65], 1.0)

                # ---- phase G: gather + exp(bias), one combined tile per t ----
                for t, (k0, pw) in enumerate(KTILES):
                    qlo = QLO[t]
                    W = WID[t]
                    bt = stagep.tile([128, WID[0] * NH], BF16, tag="btstage",
                                     name="btstage")
                    klo = max(k0, 1)
                    p0 = klo - k0
                    qg = max(qlo, 1)
                    colg = (qg - qlo) * NH
                    qw0 = (qg - 1) // GRID
                    QR = (NTOK - qg) // GRID
                    KR = (pw - p0) // GRID
                    assert (qg - 1) % GRID == 0 and (klo - 1) % GRID == 0
                    assert (NTOK - qg) % GRID == 0 and (pw - p0) % GRID == 0
                    for kr in range(KR):
                        kw = (klo - 1) // GRID + kr
                        d1_0 = qw0 - kw + (GRID - 1)
                        assert 0 <= d1_0 and d1_0 + QR <= 47
                        src = bass.AP(t3m_d.tensor, d1_0 * T3_D1,
                                      [[T3_KH, GRID], [T3_D1, QR], [1, T3_D1]])
                        dst = bt[p0 + kr * GRID:p0 + (kr + 1) * GRID,
                                 colg:colg + QR * T3_D1].rearrange(
                                     "p (qr i) -> p qr i", i=T3_D1)
                        nc.sync.dma_start(dst, src)
                    if t == 0:
                        # cls column k=0 (partition 0): constant, never masked
                        cnt = NTOK - qg
                        src = bass.AP(pe_d.tensor, (NRD - 2) * NH,
                                      [[0, 1], [0, cnt], [1, NH]])
                        dst = bt[0:1, colg:colg + cnt * NH].rearrange(
                            "p (a h) -> p a h", h=NH)
                        nc.gpsimd.dma_start(dst, src)  # casting DMA f32->bf16
                        # q=0 column: only (0,0) survives; its value cancels
                        # in the softmax normalization
                        nc.vector.memset(bt[0:pw, 0:NH], NEG)
                        nc.vector.memset(bt[0:1, 0:NH], 0.0)
                    src = bt[0:pw, 0:W * NH].rearrange("p (q h) -> p q h", h=NH)
                    dst = expb[t][0:pw, :].rearrange("p (h q) -> p q h", h=NH)
                    nc.scalar.activation(dst, src, mybir.ActivationFunctionType.Exp)

            # ======== phases B + D interleaved per head-pair ========
            wqk = [wqk_pool.tile([128, 1536], F16, tag=f"wqk{c}", name=f"wqk{c}")
                   for c in range(6)]
            for c in range(6):
                nc.sync.dma_start(wqk[c][:], qkvw_d[c * 128:(c + 1) * 128, 0:1536])

            with tc.tile_pool(name="ps_qk", bufs=4, space="PSUM") as ps_qk, \
                 tc.tile_pool(name="ps_sT", bufs=2, space="PSUM") as ps_sT, \
                 tc.tile_pool(name="ps_OT", bufs=2, space="PSUM") as ps_OT, \
                 tc.tile_pool(name="att_tmp", bufs=8) as att_tmp, \
                 tc.tile_pool(name="es_pool", bufs=10) as es_pool, \
                 tc.tile_pool(name="p_pool", bufs=10) as p_pool:
                for jp in range(6):
                    # ---- B: produce qT[jp], kT[jp] ----
                    for r in (jp, jp + 6):
                        wcol0 = r * 128
                        dst = qT[r] if r < 6 else kT[r - 6]
                        for nb0, nbw in NBLK:
                            ps = ps_qk.tile([128, 386], F32, tag="psqk",
                                            name="psqk")
                            for c in range(6):
                                nc.tensor.matmul(ps[0:128, 0:nbw],
                                                 wqk[c][:, wcol0:wcol0 + 128],
                                                 xT[c][:, nb0:nb0 + nbw],
                                                 start=(c == 0), stop=(c == 5))
                            nc.scalar.activation(
                                dst[:, nb0:nb0 + nbw], ps[0:128, 0:nbw],
                                mybir.ActivationFunctionType.Identity,
                                bias=qk_bias[r][:],
                                scale=(SCALE if r < 6 else 1.0))
                    # ---- D: attention for both batches / both q-blocks ----
                    for b in range(BLOC):
                        for (qstart, qN) in QBLOCKS:
                            qend = qstart + qN
                            valid_t = [t for t in range(5) if QLO[t] < qend]
                            tlast = valid_t[-1]
                            psO = [ps_OT.tile([65, 456], F32, tag="psOT",
                                              name="psOT") for _ in range(2)]
                            for t in valid_t:
                                k0, pw = KTILES[t]
                                qlo = max(qstart, QLO[t])
                                off = qlo - qstart
                                Nt = qend - qlo
                                ebase = qlo - QLO[t]
                                psS = [ps_sT.tile([128, 456], F32, tag="psS",
                                                  name="psS") for _ in range(2)]
                                for side in range(2):
                                    r0 = side * 64
                                    nc.tensor.matmul(
                                        psS[side][0:pw, 0:Nt],
                                        kT[jp][r0:r0 + 64,
                                               b * NTOK + k0:b * NTOK + k0 + pw],
                                        qT[jp][r0:r0 + 64,
                                               b * NTOK + qlo:b * NTOK + qlo + Nt],
                                        start=True, stop=True,
                                        tile_position=(r0, 0))
                                for side in range(2):
                                    h = 2 * jp + side
                                    es = es_pool.tile([128, 456], F16, tag="es",
                                                      name="es")
                                    nc.scalar.activation(
                                        es[0:pw, 0:Nt], psS[side][0:pw, 0:Nt],
                                        mybir.ActivationFunctionType.Exp)
                                    p = p_pool.tile([128, 456], F16, tag="p",
                                                    name="p")
                                    nc.vector.tensor_tensor(
                                        out=p[0:pw, 0:Nt],
                                        in0=es[0:pw, 0:Nt],
                                        in1=expb[t][0:pw,
                                                    h * WID[t] + ebase:
                                                    h * WID[t] + ebase + Nt],
                                        op=mybir.AluOpType.mult)
                                    nc.tensor.matmul(
                                        psO[side][0:65, off:off + Nt],
                                        vt[b][t][0:pw, h * 65:(h + 1) * 65],
                                        p[0:pw, 0:Nt],
                                        start=(t == valid_t[0]),
                                        stop=(t == tlast))
                            for side in range(2):
                                recip = att_tmp.tile([1, 456], F32, tag="recip",
                                                     name="recip")
                                nc.vector.reciprocal(recip[0:1, 0:qN],
                                                     psO[side][64:65, 0:qN])
                                rb = att_tmp.tile([64, 456], F32, tag="rb",
                                                  name="rb")
                                nc.gpsimd.partition_broadcast(rb[0:64, 0:qN],
                                                              recip[0:1, 0:qN])
                                r0 = side * 64
                                nc.vector.tensor_tensor(
                                    out=oT[jp][r0:r0 + 64,
                                               b * NTOK + qstart:b * NTOK + qend],
                                    in0=psO[side][0:64, 0:qN],
                                    in1=rb[0:64, 0:qN],
                                    op=mybir.AluOpType.mult)

        # ================= phase E: output projection =================
        with tc.tile_pool(name="wo", bufs=1) as wo, \
             tc.tile_pool(name="ps_o", bufs=3, space="PSUM") as ps_o, \
             tc.tile_pool(name="out_sb", bufs=4) as out_sb:
            wot = [wo.tile([128, CDIM], F16, tag=f"wo{c}", name=f"wo{c}")
                   for c in range(6)]
            for c in range(6):
                nc.sync.dma_start(wot[c][:], outw_d[c * 128:(c + 1) * 128, :])
            for b in range(BLOC):
                for m0 in range(0, NTOK, 128):
                    mw = min(128, NTOK - m0)
                    ot = out_sb.tile([128, CDIM], F32, tag="ot", name="ot")
                    for half in range(2):
                        ps = ps_o.tile([128, 384], F32, tag="pso", name="pso")
                        for c in range(6):
                            nc.tensor.matmul(
                                ps[0:mw, :],
                                oT[c][:, b * NTOK + m0:b * NTOK + m0 + mw],
                                wot[c][:, half * 384:(half + 1) * 384],
                                start=(c == 0), stop=(c == 5))
                        nc.vector.tensor_tensor(
                            out=ot[0:mw, half * 384:(half + 1) * 384],
                            in0=ps[0:mw, :],
                            in1=obias[0:mw, half * 384:(half + 1) * 384],
                            op=mybir.AluOpType.add)
                    nc.sync.dma_start(
                        y_d[b * NTOK + m0:b * NTOK + m0 + mw, :], ot[0:mw, :])


def kernel(x, qkv_w, qkv_b, pos_emb, out_w, out_b, rel_index):
    x = np.asarray(x, dtype=np.float32)
    qkv_w = np.asarray(qkv_w, dtype=np.float32)
    qkv_b = np.asarray(qkv_b, dtype=np.float32)
    pos_emb = np.asarray(pos_emb, dtype=np.float32)
    out_w = np.asarray(out_w, dtype=np.float32)
    out_b = np.asarray(out_b, dtype=np.float32)
    ri = np.asarray(rel_index, dtype=np.int32)

    key = ri.tobytes()
    if key not in _CACHE:
        _CACHE[key] = _build(ri)
    nc = _CACHE[key]

    t3m, pos_embT = _host_prep(pos_emb, ri)
    in_maps = []
    for c in range(NCORES):
        shard = np.ascontiguousarray(
            x[c * BLOC:(c + 1) * BLOC].reshape(NSEQ, CDIM))
        in_maps.append({
            "x_in": shard,
            "qkv_w_h": qkv_w.astype(np.float16),
            "qkv_b": qkv_b,
            "qkv_b_h": qkv_b.astype(np.float16),
            "t3m": t3m,
            "pos_embT": pos_embT,
            "out_w_h": out_w.astype(np.float16),
            "out_b_h": out_b.astype(np.float16),
        })
    res = run_bass_kernel_spmd(nc, in_maps, core_ids=list(range(NCORES)))
    out = np.empty((B, NTOK, CDIM), dtype=np.float32)
    for c in range(NCORES):
        out[c * BLOC:(c + 1) * BLOC] = res.results[c]["y"].reshape(BLOC, NTOK, CDIM)
    return out

